# revision 5
# baseline (speedup 1.0000x reference)
"""MoE-routed conditional conv kernel for Trainium2 (8 NeuronCores).

Problem: x:[64,256,32,32], 4 conv branches (k=1,3,5,7) with per-sample
branch selection (sample_arc) and a per-sample class-embedding bias
(e_b[y]).  We route: each sample's conv is computed only for its
selected branch.

Algorithm: 1D Winograd F(2,k) along the x-axis for k=3,5,7 (direct
conv for k=1).  The x-axis input transform (B^T) and the filter
transform (G) are applied host-side in fp32 and stored as bf16; the
PE computes, for each Winograd channel j, a direct conv over (ky, cin)
accumulated in PSUM; the inverse transform (A^T, 2 outputs per tile)
runs on the Vector engine as fused scalar_tensor_tensor accumulations,
and the Scalar engine adds the class-embedding bias.  This cuts PE
work on k=7 by 1.75x, k=5 1.67x, k=3 1.5x vs direct conv.

Numerics (measured vs fp64 reference, max-err / max|ref| over the
routed batch): F(2,7) pts {0,±1,±2,±1/2}: 1.02e-2; F(2,5) pts
{0,1,-1,2,-1/2}: 5.95e-3; F(2,3) pts {0,1,-1}: 2.50e-3 — all inside
the 2e-2 gate.

Distribution: SPMD over 8 cores; work unit = "slot" = (sample, band of
ro output rows); per-branch slot counts padded to a multiple of 8.
"""

import math
import sys
import types

import numpy as np

try:
    import concourse.bass as bass  # noqa: F401
except Exception:  # pragma: no cover - fallback when env lacks preloaded paths
    for p in ("/opt/trn_rl_repo", "/root/.axon_site/_ro/trn_rl_repo"):
        if p not in sys.path:
            sys.path.insert(0, p)
    import concourse.bass as bass  # noqa: F401

import ml_dtypes
import concourse.tile as tile
from concourse import bacc, mybir
from concourse import bass_utils

N_CORES = 8
NUM_BRANCH = 4
KERNEL_SIZES = (1, 3, 5, 7)
IN_C = 256
OUT_C = 256
H = W = 32
T = W // 2          # x-tiles per row (2 outputs per tile)
WARMUP_MM = 16

NDT = ml_dtypes.bfloat16
MDT = mybir.dt.bfloat16

# Winograd interpolation points per kernel size (finite points; +inf row).
WINO_POINTS = {
    3: [0.0, 1.0, -1.0],
    5: [0.0, 1.0, -1.0, 2.0, -0.5],
    7: [0.0, 1.0, -1.0, 2.0, -2.0, 0.5, -0.5],
}

# Branch emission order: small weights first (hides weight streaming),
# tiny k=1 branch last (short drain tail).
EMIT_ORDER = (1, 2, 3, 0)

_PROGRAM_CACHE = {}


def _install_profile_hook():
    name = "antenv.axon_hooks"
    if name in sys.modules:
        return
    try:
        import antenv.axon_hooks  # noqa: F401
        return
    except ImportError:
        pass
    m = types.ModuleType(name)
    holder = [None]
    m.set_axon_ntff_profile_hook = lambda h: holder.__setitem__(0, h)
    m.get_axon_ntff_profile_hook = lambda: holder[0]
    sys.modules[name] = m
    try:
        import antenv
        antenv.axon_hooks = m
        from trn_agent_boot.trn_boot import _ntff_profile_via_ctypes
        m.set_axon_ntff_profile_hook(
            _ntff_profile_via_ctypes("/opt/axon/libaxon_pjrt.so")
        )
    except Exception:
        pass


def _build_toom(m, r, points):
    """Toom-Cook/Winograd matrices: out = A.T @ ((G@g) * (Bt@d)).

    d length n=m+r-1 (correlation 'valid' producing m outputs)."""
    n = m + r - 1
    fin = [float(p) for p in points]
    assert len(fin) == n - 1
    A = np.zeros((n, m))
    for j, a in enumerate(fin):
        A[j] = [a ** i for i in range(m)]
    A[n - 1] = [0] * (m - 1) + [1]
    G = np.zeros((n, r))
    for j, a in enumerate(fin):
        Na = np.prod([a - b for b in fin if b != a])
        G[j] = [a ** i / Na for i in range(r)]
    G[n - 1] = [0] * (r - 1) + [1]
    Bt = np.zeros((n, n))
    for l in range(n):
        rows, rhs = [], []
        for kk in range(r):
            c = np.zeros(m)
            if 0 <= l - kk < m:
                c[l - kk] = 1.0
            rows.append(A.T * G[:, kk][None, :])
            rhs.append(c)
        beta, _, _, _ = np.linalg.lstsq(np.vstack(rows), np.concatenate(rhs),
                                        rcond=None)
        Bt[:, l] = beta
    return A, G, Bt


_TOOM = {k: _build_toom(2, k, WINO_POINTS[k]) for k in (3, 5, 7)}


def _branch_cfg(count, k):
    """Pick output rows per slot (16 or 8) minimizing padding waste."""
    best = None
    for ro in (16, 8):
        units = (H // ro) * count
        slots = int(math.ceil(units / N_CORES))
        waste = (slots * N_CORES - units) * ro
        key = (waste, slots)
        if best is None or key < best[0]:
            best = (key, ro, slots)
    _, ro, slots = best
    return ro, slots


def _build_program(cfg):
    """cfg: tuple over branches of (k, n_slots, ro)."""
    if cfg in _PROGRAM_CACHE:
        return _PROGRAM_CACHE[cfg]

    nc = bacc.Bacc("TRN2", target_bir_lowering=False, debug=False,
                   num_devices=N_CORES)
    n_total = sum(n for _, n, _ in cfg)

    x_d, w_d, out_d = {}, {}, {}
    for b in EMIT_ORDER:
        k, n, ro = cfg[b]
        if n == 0:
            continue
        c = k // 2
        if k == 1:
            x_d[b] = nc.dram_tensor(f"x{b}", [128, n, 2, ro, W], MDT,
                                    kind="ExternalInput").ap()
            w_d[b] = nc.dram_tensor(f"w{b}", [128, 4 * 128], MDT,
                                    kind="ExternalInput").ap()
            out_d[b] = nc.dram_tensor(f"out{b}", [n, 128, 2 * ro * W],
                                      mybir.dt.float32,
                                      kind="ExternalOutput").ap()
        else:
            nj = k + 1
            rows = ro + 2 * c
            x_d[b] = nc.dram_tensor(f"x{b}", [128, n, 2, rows, nj, T], MDT,
                                    kind="ExternalInput").ap()
            w_d[b] = nc.dram_tensor(f"w{b}", [128, nj * k * 4 * 128], MDT,
                                    kind="ExternalInput").ap()
            out_d[b] = nc.dram_tensor(f"out{b}", [n, 128, 4 * ro * T],
                                      mybir.dt.float32,
                                      kind="ExternalOutput").ap()
    emb_d = nc.dram_tensor("emb", [128, n_total * 2], mybir.dt.float32,
                           kind="ExternalInput").ap()

    from contextlib import ExitStack
    with tile.TileContext(nc) as tc:
        with ExitStack() as ctx:
            wpool = ctx.enter_context(tc.tile_pool(name="wpool", bufs=1))
            xpool = ctx.enter_context(tc.tile_pool(name="xpool", bufs=2))
            apool = ctx.enter_context(tc.tile_pool(name="apool", bufs=2))
            spool = ctx.enter_context(tc.tile_pool(name="spool", bufs=2))
            epool = ctx.enter_context(tc.tile_pool(name="epool", bufs=1))
            ppool = ctx.enter_context(
                tc.tile_pool(name="ppool", bufs=8, space="PSUM"))

            emb_t = epool.tile([128, n_total * 2], mybir.dt.float32,
                               tag="emb")
            nc.scalar.dma_start(emb_t[:], emb_d[:])

            # PE warm-up while first DMAs stream (lifts the clock p-state).
            dummy = epool.tile([128, 128], MDT, tag="dummy")
            nc.vector.memset(dummy[:], 0.0)
            wps = ppool.tile([128, 128], mybir.dt.float32, tag="acc",
                             name="warm_psum")
            for _ in range(WARMUP_MM):
                nc.tensor.matmul(wps[:], dummy[:], dummy[:],
                                 start=True, stop=True)

            # Resident weights; stream per-j chunks on the gpsimd queue in
            # emission order so each branch's weights land before its slots.
            w_t = {}
            for b in EMIT_ORDER:
                k, n, ro = cfg[b]
                if n == 0:
                    continue
                if k == 1:
                    wt = wpool.tile([128, 4 * 128], MDT, tag=f"w{b}")
                    nc.gpsimd.dma_start(wt[:], w_d[b][:])
                else:
                    nj = k + 1
                    chunk = k * 4 * 128
                    wt = wpool.tile([128, nj * chunk], MDT, tag=f"w{b}")
                    for j in range(nj):
                        nc.gpsimd.dma_start(
                            wt[:, j * chunk:(j + 1) * chunk],
                            w_d[b][:, j * chunk:(j + 1) * chunk])
                w_t[b] = wt

            mult = mybir.AluOpType.mult
            addop = mybir.AluOpType.add
            # sync queue is reserved for x-input DMAs: an out-DMA trigger
            # waiting on the bias-add would head-of-line-block the next
            # slot's input prefetch.
            out_queues = [nc.gpsimd, nc.scalar]
            out_i = 0
            xbufs = {0: 2, 1: 2, 2: 3, 3: 3}
            slot_base = 0
            for b in EMIT_ORDER:
                k, n, ro = cfg[b]
                if n == 0:
                    continue
                c = k // 2
                wt = w_t[b]
                if k == 1:
                    nf = ro * W
                    for i in range(n):
                        xt = xpool.tile([128, 2, ro, W], MDT, tag=f"x{b}",
                                        bufs=xbufs[b], name=f"x{b}_{i}")
                        nc.sync.dma_start(xt[:], x_d[b][:, i])
                        st = spool.tile([128, 2 * nf], mybir.dt.float32,
                                        tag=f"st{b}", name=f"st{b}_{i}")
                        col = (slot_base + i) * 2
                        for oc in range(2):
                            ps = ppool.tile([128, nf], mybir.dt.float32,
                                            tag="acc", name=f"ps{b}_{i}_{oc}")
                            for ic in range(2):
                                lhsT = wt[:, (ic * 2 + oc) * 128:
                                          (ic * 2 + oc + 1) * 128]
                                nc.tensor.matmul(ps[:], lhsT, xt[:, ic],
                                                 start=(ic == 0),
                                                 stop=(ic == 1))
                            nc.scalar.add(st[:, oc * nf:(oc + 1) * nf],
                                          ps[:],
                                          emb_t[:, col + oc:col + oc + 1])
                        q = out_queues[out_i % len(out_queues)]
                        q.dma_start(out_d[b][i], st[:])
                        out_i += 1
                    slot_base += n
                    continue

                nj = k + 1
                rows = ro + 2 * c
                nf = ro * T
                A = _TOOM[k][0]
                for i in range(n):
                    xt = xpool.tile([128, 2, rows, nj, T], MDT, tag=f"x{b}",
                                    bufs=xbufs[b], name=f"x{b}_{i}")
                    nc.sync.dma_start(xt[:], x_d[b][:, i])
                    accs = {}
                    for j in range(nj):
                        for oc in range(2):
                            ps = ppool.tile([128, nf], mybir.dt.float32,
                                            tag="acc",
                                            name=f"ps{b}_{i}_{j}_{oc}")
                            base = ((j * k) * 2) * 2 * 128
                            for ky in range(k):
                                for ic in range(2):
                                    o = (((j * k + ky) * 2 + ic) * 2
                                         + oc) * 128
                                    lhsT = wt[:, o:o + 128]
                                    rhs = xt[:, ic, ky:ky + ro, j, :]
                                    nc.tensor.matmul(
                                        ps[:], lhsT, rhs,
                                        start=(ky == 0 and ic == 0),
                                        stop=(ky == k - 1 and ic == 1))
                            for bb in range(2):
                                coef = float(A[j, bb])
                                if coef == 0.0:
                                    continue
                                prev = accs.get((bb, oc))
                                na = apool.tile([128, nf], mybir.dt.float32,
                                                tag=f"acc{bb}{oc}",
                                                name=f"a{b}_{i}_{j}_{bb}{oc}")
                                if prev is None:
                                    nc.vector.tensor_scalar_mul(
                                        na[:], ps[:], coef)
                                else:
                                    nc.vector.scalar_tensor_tensor(
                                        na[:], ps[:], coef, prev[:],
                                        op0=mult, op1=addop)
                                accs[(bb, oc)] = na
                    st = spool.tile([128, 4 * nf], mybir.dt.float32,
                                    tag=f"st{b}", name=f"st{b}_{i}")
                    col = (slot_base + i) * 2
                    for oc in range(2):
                        for bb in range(2):
                            nc.scalar.add(
                                st[:, (oc * 2 + bb) * nf:
                                   (oc * 2 + bb + 1) * nf],
                                accs[(bb, oc)][:],
                                emb_t[:, col + oc:col + oc + 1])
                    q = out_queues[out_i % len(out_queues)]
                    q.dma_start(out_d[b][i], st[:])
                    out_i += 1
                slot_base += n

    nc.finalize()
    _PROGRAM_CACHE[cfg] = nc
    return nc


def _prepare(inputs):
    x = np.asarray(inputs["x"], dtype=np.float32)
    y = np.asarray(inputs["y"]).astype(np.int64)
    arc = np.asarray(inputs["sample_arc"]).astype(np.int64)
    ws = [np.asarray(inputs[f"w{i}"], dtype=np.float32) for i in range(4)]
    es = [np.asarray(inputs[f"e{i}"], dtype=np.float32) for i in range(4)]
    B = x.shape[0]

    counts = np.bincount(arc, minlength=NUM_BRANCH)
    cfg = []
    for b in range(NUM_BRANCH):
        ro, slots = _branch_cfg(int(counts[b]), KERNEL_SIZES[b])
        cfg.append((KERNEL_SIZES[b], slots, ro))
    cfg = tuple(cfg)
    n_total = sum(n for _, n, _ in cfg)

    # per-branch slot assignment: 8*n_b entries of (sample, band) or None
    assign = {}
    for b in range(NUM_BRANCH):
        k, n, ro = cfg[b]
        bands = H // ro
        units = [(s, u) for s in range(B) if arc[s] == b
                 for u in range(bands)]
        units += [None] * (N_CORES * n - len(units))
        assign[b] = units

    # ---- weights ----
    w_arrs = {}
    for b in range(NUM_BRANCH):
        k, n, ro = cfg[b]
        if n == 0:
            continue
        w6 = ws[b].reshape(2, 128, 2, 128, k, k)  # oc,m,ic,p,ky,kx
        if k == 1:
            # [p, ic, oc, m]
            wt = np.ascontiguousarray(
                w6[:, :, :, :, 0, 0].transpose(3, 2, 0, 1))
            w_arrs[b] = wt.reshape(128, 4 * 128).astype(NDT)
        else:
            G = _TOOM[k][1]
            nj = k + 1
            # wt[p, j, ky, ic, oc, m] = sum_kx G[j,kx] w6[oc,m,ic,p,ky,kx]
            wt = np.einsum("jx,omipyx->pjyiom", G.astype(np.float32), w6)
            w_arrs[b] = np.ascontiguousarray(wt).reshape(
                128, nj * k * 4 * 128).astype(NDT)

    # ---- per-branch full transformed inputs ----
    # xr: [B, 128p, 2ic, H, W]
    xr = x.reshape(B, 2, 128, H, W).transpose(0, 2, 1, 3, 4)
    xw_full = {}
    for b in range(NUM_BRANCH):
        k, n, ro = cfg[b]
        if n == 0 or k == 1:
            continue
        c = k // 2
        nj = k + 1
        Bt = _TOOM[k][2].astype(np.float32)
        sel = np.where(arc == b)[0]
        S = len(sel)
        xp = np.zeros((S, 128, 2, H + 2 * c, W + 2 * c), np.float32)
        xp[:, :, :, c:c + H, c:c + W] = xr[sel]
        D = np.empty((S, 128, 2, H + 2 * c, T, nj), np.float32)
        for t in range(T):
            D[:, :, :, :, t, :] = xp[:, :, :, :, 2 * t:2 * t + nj]
        # Xw: [S, 128, 2, rows_full, nj, T]
        Xw = np.einsum("ju,spcrtu->spcrjt", Bt, D).astype(NDT)
        xw_full[b] = (sel, Xw)

    in_maps = []
    meta = []
    for core in range(N_CORES):
        im = {}
        slots = []
        emb_arr = np.zeros((128, n_total * 2), dtype=np.float32)
        idx = 0
        for b in EMIT_ORDER:
            k, n, ro = cfg[b]
            if n == 0:
                continue
            c = k // 2
            if k == 1:
                xa = np.zeros((128, n, 2, ro, W), dtype=NDT)
                for i in range(n):
                    hs = assign[b][core * n + i]
                    if hs is not None:
                        s, u = hs
                        xa[:, i] = xr[s, :, :, u * ro:(u + 1) * ro, :]
                        ev = es[b][y[s]]
                        emb_arr[:, (idx + i) * 2 + 0] = ev[:128]
                        emb_arr[:, (idx + i) * 2 + 1] = ev[128:]
                        slots.append((b, i, s, u, ro))
            else:
                nj = k + 1
                rows = ro + 2 * c
                sel, Xw = xw_full[b]
                pos = {s: p for p, s in enumerate(sel)}
                xa = np.zeros((128, n, 2, rows, nj, T), dtype=NDT)
                for i in range(n):
                    hs = assign[b][core * n + i]
                    if hs is not None:
                        s, u = hs
                        xa[:, i] = Xw[pos[s], :, :,
                                      u * ro:u * ro + rows]
                        ev = es[b][y[s]]
                        emb_arr[:, (idx + i) * 2 + 0] = ev[:128]
                        emb_arr[:, (idx + i) * 2 + 1] = ev[128:]
                        slots.append((b, i, s, u, ro))
            im[f"x{b}"] = xa
            im[f"w{b}"] = w_arrs[b]
            idx += n
        im["emb"] = emb_arr
        in_maps.append(im)
        meta.append(slots)

    return cfg, in_maps, meta


def _assemble(results, meta, B):
    out = np.zeros((B, OUT_C, H, W), dtype=np.float32)
    for core in range(N_CORES):
        r = results[core]
        for b, i, s, u, ro in meta[core]:
            blk = r[f"out{b}"][i]
            if KERNEL_SIZES[b] == 1:
                o = blk.reshape(128, 2, ro, W).transpose(1, 0, 2, 3)
                out[s, :, u * ro:(u + 1) * ro, :] = o.reshape(OUT_C, ro, W)
            else:
                o = blk.reshape(128, 2, 2, ro, T).transpose(1, 0, 3, 4, 2)
                out[s, :, u * ro:(u + 1) * ro, :] = o.reshape(OUT_C, ro, W)
    return out


def run(inputs, trace=False):
    if trace:
        _install_profile_hook()
    cfg, in_maps, meta = _prepare(inputs)
    nc = _build_program(cfg)
    res = bass_utils.run_bass_kernel_spmd(
        nc, in_maps, core_ids=list(range(N_CORES)), trace=trace)
    B = int(np.asarray(inputs["x"]).shape[0])
    out = _assemble(res.results, meta, B)
    return out, res


def kernel(**inputs):
    out, _ = run(inputs, trace=False)
    return out


# revision 8
# speedup vs baseline: 1.1228x; 1.1228x over previous
"""MoE-routed conditional conv kernel for Trainium2 (8 NeuronCores).

Problem: x:[64,256,32,32], 4 conv branches (k=1,3,5,7) with per-sample
branch selection (sample_arc) and a per-sample class-embedding bias
(e_b[y]).  We route: each sample's conv is computed only for its
selected branch.

Algorithm: 1D Winograd F(2,k) along the x-axis for k=3,5,7 (direct
conv for k=1).  The x-axis input transform (B^T) and the filter
transform (G) are applied host-side in fp32 and stored as bf16; the
PE computes, for each Winograd channel j, a direct conv over (ky, cin)
accumulated in PSUM; the inverse transform (A^T, 2 outputs per tile)
runs on the Vector engine as fused scalar_tensor_tensor accumulations,
and the Scalar engine adds the class-embedding bias.  This cuts PE
work on k=7 by 1.75x, k=5 1.67x, k=3 1.5x vs direct conv.

Numerics (measured vs fp64 reference, max-err / max|ref| over the
routed batch): F(2,7) pts {0,±1,±2,±1/2}: 1.02e-2; F(2,5) pts
{0,1,-1,2,-1/2}: 5.95e-3; F(2,3) pts {0,1,-1}: 2.50e-3 — all inside
the 2e-2 gate.

Distribution: SPMD over 8 cores; work unit = "slot" = (sample, band of
ro output rows); per-branch slot counts padded to a multiple of 8.
"""

import math
import sys
import types

import numpy as np

try:
    import concourse.bass as bass  # noqa: F401
except Exception:  # pragma: no cover - fallback when env lacks preloaded paths
    for p in ("/opt/trn_rl_repo", "/root/.axon_site/_ro/trn_rl_repo"):
        if p not in sys.path:
            sys.path.insert(0, p)
    import concourse.bass as bass  # noqa: F401

import ml_dtypes
import concourse.tile as tile
from concourse import bacc, mybir
from concourse import bass_utils

N_CORES = 8
NUM_BRANCH = 4
KERNEL_SIZES = (1, 3, 5, 7)
IN_C = 256
OUT_C = 256
H = W = 32
T = W // 2          # x-tiles per row (2 outputs per tile)
WARMUP_MM = 32

NDT = ml_dtypes.bfloat16
MDT = mybir.dt.bfloat16

# Winograd interpolation points per kernel size (finite points; +inf row).
WINO_POINTS = {
    3: [0.0, 1.0, -1.0],
    5: [0.0, 1.0, -1.0, 2.0, -0.5],
    7: [0.0, 1.0, -1.0, 2.0, -2.0, 0.5, -0.5],
}

# Branch emission order: small weights first (hides weight streaming),
# tiny k=1 branch last (short drain tail).
EMIT_ORDER = (1, 2, 3, 0)

_PROGRAM_CACHE = {}


def _install_profile_hook():
    name = "antenv.axon_hooks"
    if name in sys.modules:
        return
    try:
        import antenv.axon_hooks  # noqa: F401
        return
    except ImportError:
        pass
    m = types.ModuleType(name)
    holder = [None]
    m.set_axon_ntff_profile_hook = lambda h: holder.__setitem__(0, h)
    m.get_axon_ntff_profile_hook = lambda: holder[0]
    sys.modules[name] = m
    try:
        import antenv
        antenv.axon_hooks = m
        from trn_agent_boot.trn_boot import _ntff_profile_via_ctypes
        m.set_axon_ntff_profile_hook(
            _ntff_profile_via_ctypes("/opt/axon/libaxon_pjrt.so")
        )
    except Exception:
        pass


def _build_toom(m, r, points):
    """Toom-Cook/Winograd matrices: out = A.T @ ((G@g) * (Bt@d)).

    d length n=m+r-1 (correlation 'valid' producing m outputs)."""
    n = m + r - 1
    fin = [float(p) for p in points]
    assert len(fin) == n - 1
    A = np.zeros((n, m))
    for j, a in enumerate(fin):
        A[j] = [a ** i for i in range(m)]
    A[n - 1] = [0] * (m - 1) + [1]
    G = np.zeros((n, r))
    for j, a in enumerate(fin):
        Na = np.prod([a - b for b in fin if b != a])
        G[j] = [a ** i / Na for i in range(r)]
    G[n - 1] = [0] * (r - 1) + [1]
    Bt = np.zeros((n, n))
    for l in range(n):
        rows, rhs = [], []
        for kk in range(r):
            c = np.zeros(m)
            if 0 <= l - kk < m:
                c[l - kk] = 1.0
            rows.append(A.T * G[:, kk][None, :])
            rhs.append(c)
        beta, _, _, _ = np.linalg.lstsq(np.vstack(rows), np.concatenate(rhs),
                                        rcond=None)
        Bt[:, l] = beta
    return A, G, Bt


_TOOM = {k: _build_toom(2, k, WINO_POINTS[k]) for k in (3, 5, 7)}


def _branch_cfg(count, k):
    """Pick output rows per slot (16 or 8) minimizing padding waste.

    k=3 stays at ro=16: its ro=8 inverse-transform ops are [128,128] and
    overhead-dominated, making the Vector engine slower than the PE per
    slot (PSUM-drain bound)."""
    if k == 3:
        units = 2 * count
        return 16, int(math.ceil(units / N_CORES))
    best = None
    for ro in (16, 8):
        units = (H // ro) * count
        slots = int(math.ceil(units / N_CORES))
        waste = (slots * N_CORES - units) * ro
        key = (waste, slots)
        if best is None or key < best[0]:
            best = (key, ro, slots)
    _, ro, slots = best
    return ro, slots


def _build_program(cfg):
    """cfg: tuple over branches of (k, n_slots, ro)."""
    if cfg in _PROGRAM_CACHE:
        return _PROGRAM_CACHE[cfg]

    nc = bacc.Bacc("TRN2", target_bir_lowering=False, debug=False,
                   num_devices=N_CORES)
    n_total = sum(n for _, n, _ in cfg)

    x_d, w_d, out_d = {}, {}, {}
    for b in EMIT_ORDER:
        k, n, ro = cfg[b]
        if n == 0:
            continue
        c = k // 2
        if k == 1:
            x_d[b] = nc.dram_tensor(f"x{b}", [128, n, 2, ro, W], MDT,
                                    kind="ExternalInput").ap()
            w_d[b] = nc.dram_tensor(f"w{b}", [128, 4 * 128], MDT,
                                    kind="ExternalInput").ap()
            out_d[b] = nc.dram_tensor(f"out{b}", [n, 128, 2 * ro * W],
                                      mybir.dt.float32,
                                      kind="ExternalOutput").ap()
        else:
            nj = k + 1
            rows = ro + 2 * c
            x_d[b] = nc.dram_tensor(f"x{b}", [128, n, 2, rows, nj, T], MDT,
                                    kind="ExternalInput").ap()
            w_d[b] = nc.dram_tensor(f"w{b}", [128, nj * k * 4 * 128], MDT,
                                    kind="ExternalInput").ap()
            out_d[b] = nc.dram_tensor(f"out{b}", [n, 128, 4 * ro * T],
                                      mybir.dt.float32,
                                      kind="ExternalOutput").ap()
    emb_d = nc.dram_tensor("emb", [128, n_total * 2], mybir.dt.float32,
                           kind="ExternalInput").ap()

    from contextlib import ExitStack
    with tile.TileContext(nc) as tc:
        with ExitStack() as ctx:
            wpool = ctx.enter_context(tc.tile_pool(name="wpool", bufs=1))
            xpool = ctx.enter_context(tc.tile_pool(name="xpool", bufs=2))
            apool = ctx.enter_context(tc.tile_pool(name="apool", bufs=2))
            spool = ctx.enter_context(tc.tile_pool(name="spool", bufs=2))
            epool = ctx.enter_context(tc.tile_pool(name="epool", bufs=1))
            ppool = ctx.enter_context(
                tc.tile_pool(name="ppool", bufs=8, space="PSUM"))

            emb_t = epool.tile([128, n_total * 2], mybir.dt.float32,
                               tag="emb")
            nc.scalar.dma_start(emb_t[:], emb_d[:])

            # PE warm-up while first DMAs stream (lifts the clock p-state).
            dummy = epool.tile([128, 128], MDT, tag="dummy")
            nc.vector.memset(dummy[:], 0.0)
            wps = ppool.tile([128, 128], mybir.dt.float32, tag="acc",
                             name="warm_psum")
            for _ in range(WARMUP_MM):
                nc.tensor.matmul(wps[:], dummy[:], dummy[:],
                                 start=True, stop=True)

            # Resident weights; stream per-j chunks on the gpsimd queue in
            # emission order so each branch's weights land before its slots.
            w_t = {}
            for b in EMIT_ORDER:
                k, n, ro = cfg[b]
                if n == 0:
                    continue
                if k == 1:
                    wt = wpool.tile([128, 4 * 128], MDT, tag=f"w{b}")
                    nc.gpsimd.dma_start(wt[:], w_d[b][:])
                else:
                    nj = k + 1
                    chunk = k * 4 * 128
                    wt = wpool.tile([128, nj * chunk], MDT, tag=f"w{b}")
                    for j in range(nj):
                        nc.gpsimd.dma_start(
                            wt[:, j * chunk:(j + 1) * chunk],
                            w_d[b][:, j * chunk:(j + 1) * chunk])
                w_t[b] = wt

            mult = mybir.AluOpType.mult
            addop = mybir.AluOpType.add
            # Queue roles: sync = x-input only, gpsimd = weights only,
            # scalar = outputs (each out trigger directly follows its
            # producing ACTIVATE, so no head-of-line blocking).  Sharing
            # outputs with the weight queue stalls early slots behind the
            # 13MB weight prefetch.
            out_queues = [nc.scalar]
            out_i = 0
            xbufs = {0: 2, 1: 2, 2: 2, 3: 2}
            slot_base = 0
            for b in EMIT_ORDER:
                k, n, ro = cfg[b]
                if n == 0:
                    continue
                c = k // 2
                wt = w_t[b]
                if k == 1:
                    nf = ro * W
                    for i in range(n):
                        xt = xpool.tile([128, 2, ro, W], MDT, tag=f"x{b}",
                                        bufs=xbufs[b], name=f"x{b}_{i}")
                        nc.sync.dma_start(xt[:], x_d[b][:, i])
                        st = spool.tile([128, 2 * nf], mybir.dt.float32,
                                        tag=f"st{b}", name=f"st{b}_{i}")
                        col = (slot_base + i) * 2
                        for oc in range(2):
                            ps = ppool.tile([128, nf], mybir.dt.float32,
                                            tag="acc", name=f"ps{b}_{i}_{oc}")
                            for ic in range(2):
                                lhsT = wt[:, (ic * 2 + oc) * 128:
                                          (ic * 2 + oc + 1) * 128]
                                nc.tensor.matmul(ps[:], lhsT, xt[:, ic],
                                                 start=(ic == 0),
                                                 stop=(ic == 1))
                            nc.scalar.add(st[:, oc * nf:(oc + 1) * nf],
                                          ps[:],
                                          emb_t[:, col + oc:col + oc + 1])
                        q = out_queues[out_i % len(out_queues)]
                        q.dma_start(out_d[b][i], st[:])
                        out_i += 1
                    slot_base += n
                    continue

                nj = k + 1
                rows = ro + 2 * c
                nf = ro * T
                A = _TOOM[k][0]
                for i in range(n):
                    xt = xpool.tile([128, 2, rows, nj, T], MDT, tag=f"x{b}",
                                    bufs=xbufs[b], name=f"x{b}_{i}")
                    nc.sync.dma_start(xt[:], x_d[b][:, i])
                    accs = {}
                    for j in range(nj):
                        for oc in range(2):
                            ps = ppool.tile([128, nf], mybir.dt.float32,
                                            tag="acc",
                                            name=f"ps{b}_{i}_{j}_{oc}")
                            base = ((j * k) * 2) * 2 * 128
                            for ky in range(k):
                                for ic in range(2):
                                    o = (((j * k + ky) * 2 + ic) * 2
                                         + oc) * 128
                                    lhsT = wt[:, o:o + 128]
                                    rhs = xt[:, ic, ky:ky + ro, j, :]
                                    nc.tensor.matmul(
                                        ps[:], lhsT, rhs,
                                        start=(ky == 0 and ic == 0),
                                        stop=(ky == k - 1 and ic == 1))
                            for bb in range(2):
                                coef = float(A[j, bb])
                                if coef == 0.0:
                                    continue
                                prev = accs.get((bb, oc))
                                na = apool.tile([128, nf], mybir.dt.float32,
                                                tag=f"acc{bb}{oc}", bufs=3,
                                                name=f"a{b}_{i}_{j}_{bb}{oc}")
                                if prev is None:
                                    nc.vector.tensor_scalar_mul(
                                        na[:], ps[:], coef)
                                else:
                                    nc.vector.scalar_tensor_tensor(
                                        na[:], ps[:], coef, prev[:],
                                        op0=mult, op1=addop)
                                accs[(bb, oc)] = na
                    st = spool.tile([128, 4 * nf], mybir.dt.float32,
                                    tag=f"st{b}", name=f"st{b}_{i}")
                    col = (slot_base + i) * 2
                    for oc in range(2):
                        for bb in range(2):
                            nc.scalar.add(
                                st[:, (oc * 2 + bb) * nf:
                                   (oc * 2 + bb + 1) * nf],
                                accs[(bb, oc)][:],
                                emb_t[:, col + oc:col + oc + 1])
                    q = out_queues[out_i % len(out_queues)]
                    q.dma_start(out_d[b][i], st[:])
                    out_i += 1
                slot_base += n

    nc.finalize()
    _PROGRAM_CACHE[cfg] = nc
    return nc


def _prepare(inputs):
    x = np.asarray(inputs["x"], dtype=np.float32)
    y = np.asarray(inputs["y"]).astype(np.int64)
    arc = np.asarray(inputs["sample_arc"]).astype(np.int64)
    ws = [np.asarray(inputs[f"w{i}"], dtype=np.float32) for i in range(4)]
    es = [np.asarray(inputs[f"e{i}"], dtype=np.float32) for i in range(4)]
    B = x.shape[0]

    counts = np.bincount(arc, minlength=NUM_BRANCH)
    cfg = []
    for b in range(NUM_BRANCH):
        ro, slots = _branch_cfg(int(counts[b]), KERNEL_SIZES[b])
        cfg.append((KERNEL_SIZES[b], slots, ro))
    cfg = tuple(cfg)
    n_total = sum(n for _, n, _ in cfg)

    # per-branch slot assignment: 8*n_b entries of (sample, band) or None
    assign = {}
    for b in range(NUM_BRANCH):
        k, n, ro = cfg[b]
        bands = H // ro
        units = [(s, u) for s in range(B) if arc[s] == b
                 for u in range(bands)]
        units += [None] * (N_CORES * n - len(units))
        assign[b] = units

    # ---- weights ----
    w_arrs = {}
    for b in range(NUM_BRANCH):
        k, n, ro = cfg[b]
        if n == 0:
            continue
        w6 = ws[b].reshape(2, 128, 2, 128, k, k)  # oc,m,ic,p,ky,kx
        if k == 1:
            # [p, ic, oc, m]
            wt = np.ascontiguousarray(
                w6[:, :, :, :, 0, 0].transpose(3, 2, 0, 1))
            w_arrs[b] = wt.reshape(128, 4 * 128).astype(NDT)
        else:
            G = _TOOM[k][1]
            nj = k + 1
            # wt[p, j, ky, ic, oc, m] = sum_kx G[j,kx] w6[oc,m,ic,p,ky,kx]
            wt = np.einsum("jx,omipyx->pjyiom", G.astype(np.float32), w6)
            w_arrs[b] = np.ascontiguousarray(wt).reshape(
                128, nj * k * 4 * 128).astype(NDT)

    # ---- per-branch full transformed inputs ----
    # xr: [B, 128p, 2ic, H, W]
    xr = x.reshape(B, 2, 128, H, W).transpose(0, 2, 1, 3, 4)
    xw_full = {}
    for b in range(NUM_BRANCH):
        k, n, ro = cfg[b]
        if n == 0 or k == 1:
            continue
        c = k // 2
        nj = k + 1
        Bt = _TOOM[k][2].astype(np.float32)
        sel = np.where(arc == b)[0]
        S = len(sel)
        xp = np.zeros((S, 128, 2, H + 2 * c, W + 2 * c), np.float32)
        xp[:, :, :, c:c + H, c:c + W] = xr[sel]
        D = np.empty((S, 128, 2, H + 2 * c, T, nj), np.float32)
        for t in range(T):
            D[:, :, :, :, t, :] = xp[:, :, :, :, 2 * t:2 * t + nj]
        # Xw: [S, 128, 2, rows_full, nj, T]
        Xw = np.einsum("ju,spcrtu->spcrjt", Bt, D).astype(NDT)
        xw_full[b] = (sel, Xw)

    in_maps = []
    meta = []
    for core in range(N_CORES):
        im = {}
        slots = []
        emb_arr = np.zeros((128, n_total * 2), dtype=np.float32)
        idx = 0
        for b in EMIT_ORDER:
            k, n, ro = cfg[b]
            if n == 0:
                continue
            c = k // 2
            if k == 1:
                xa = np.zeros((128, n, 2, ro, W), dtype=NDT)
                for i in range(n):
                    hs = assign[b][core * n + i]
                    if hs is not None:
                        s, u = hs
                        xa[:, i] = xr[s, :, :, u * ro:(u + 1) * ro, :]
                        ev = es[b][y[s]]
                        emb_arr[:, (idx + i) * 2 + 0] = ev[:128]
                        emb_arr[:, (idx + i) * 2 + 1] = ev[128:]
                        slots.append((b, i, s, u, ro))
            else:
                nj = k + 1
                rows = ro + 2 * c
                sel, Xw = xw_full[b]
                pos = {s: p for p, s in enumerate(sel)}
                xa = np.zeros((128, n, 2, rows, nj, T), dtype=NDT)
                for i in range(n):
                    hs = assign[b][core * n + i]
                    if hs is not None:
                        s, u = hs
                        xa[:, i] = Xw[pos[s], :, :,
                                      u * ro:u * ro + rows]
                        ev = es[b][y[s]]
                        emb_arr[:, (idx + i) * 2 + 0] = ev[:128]
                        emb_arr[:, (idx + i) * 2 + 1] = ev[128:]
                        slots.append((b, i, s, u, ro))
            im[f"x{b}"] = xa
            im[f"w{b}"] = w_arrs[b]
            idx += n
        im["emb"] = emb_arr
        in_maps.append(im)
        meta.append(slots)

    return cfg, in_maps, meta


def _assemble(results, meta, B):
    out = np.zeros((B, OUT_C, H, W), dtype=np.float32)
    for core in range(N_CORES):
        r = results[core]
        for b, i, s, u, ro in meta[core]:
            blk = r[f"out{b}"][i]
            if KERNEL_SIZES[b] == 1:
                o = blk.reshape(128, 2, ro, W).transpose(1, 0, 2, 3)
                out[s, :, u * ro:(u + 1) * ro, :] = o.reshape(OUT_C, ro, W)
            else:
                o = blk.reshape(128, 2, 2, ro, T).transpose(1, 0, 3, 4, 2)
                out[s, :, u * ro:(u + 1) * ro, :] = o.reshape(OUT_C, ro, W)
    return out


def run(inputs, trace=False):
    if trace:
        _install_profile_hook()
    cfg, in_maps, meta = _prepare(inputs)
    nc = _build_program(cfg)
    res = bass_utils.run_bass_kernel_spmd(
        nc, in_maps, core_ids=list(range(N_CORES)), trace=trace)
    B = int(np.asarray(inputs["x"]).shape[0])
    out = _assemble(res.results, meta, B)
    return out, res


def kernel(**inputs):
    out, _ = run(inputs, trace=False)
    return out
